# revision 1
# baseline (speedup 1.0000x reference)
"""Trainium2 Bass kernel for nn_BoundaryExtractionModule.

Data-parallel over batch: 8 samples -> 8 NeuronCores, one sample per core.

Per-core pipeline (channel-major layout [C, N] with C=64 on partitions):
  conv3x3(W_std)+depthwise-Laplacian   : 9 shift-matmuls per 512-col chunk
                                         (Laplacian folded into the taps on host)
  3-scale pooled non-local attention   : for each scale s in (4, 2, 1):
      A: row-max of logits  S = f^T f   (fp16 matmuls, DVE reduce_max)
      B: recompute S^T with the shift folded in via an augmented
         contraction row (K=65):  S'[m,q] = sum_k f_a[k,m] g_a[k,q]
         where f_a = [f; 1], g_a = [f; -rowmax]
      exp on ACT (PSUM -> fp16 SBUF)   : E^T tiles
      C: PV matmul with ones-column    : G = [f; 1] @ E^T  ->  G[64] = softmax denom
      D: out = G[0:64] * (1/G[64])     : gpsimd partition_broadcast + multiply
  bilinear x2/x4 upsample (half-pixel) : strided ops on edge-padded buffers
  residual add + DMA out.

The emission order interleaves the small scales and upsampling into scale-1's
superblock stream so every engine stays busy (Tile schedules greedily in
program order).
"""

import numpy as np

import concourse.bass as bass
import concourse.mybir as mybir
import concourse.tile as tile
from concourse import bacc
from concourse.bass_utils import run_bass_kernel_spmd
from concourse.masks import make_identity

dt = mybir.dt
AF = mybir.ActivationFunctionType
ALU = mybir.AluOpType
AX = mybir.AxisListType

C = 64
H = W = 64
N1 = H * W          # 4096
PAD = 66            # padded row length for conv
NCORES = 8

_cache = {}


def _v(ap, off, dims):
    """View of `ap` at free-offset `off` with free dims `dims` (keeps partition dim)."""
    return bass.AP(ap.tensor, ap.offset + off, [list(ap.ap[0])] + [list(d) for d in dims])


def _chunks(total, size):
    out = []
    off = 0
    while off < total:
        out.append((off, min(size, total - off)))
        off += size
    return out


def _build_nc():
    nc = bacc.Bacc(None, target_bir_lowering=False)
    xp_d = nc.dram_tensor("xp", [C, PAD * PAD], dt.float16, kind="ExternalInput")
    wt_d = nc.dram_tensor("wt", [C, 9 * C], dt.float16, kind="ExternalInput")
    out_d = nc.dram_tensor("out", [C, N1], dt.float32, kind="ExternalOutput")

    with tile.TileContext(nc) as tc:
        with (
            tc.tile_pool(name="sb", bufs=1) as sb,
            tc.tile_pool(name="ga", bufs=4) as ga_pool,
            tc.tile_pool(name="et", bufs=6) as et_pool,
            tc.tile_pool(name="dd", bufs=4) as dd_pool,
            tc.tile_pool(name="cm", bufs=18) as cm_pool,
            tc.tile_pool(name="aa", bufs=3, space="PSUM") as aa,
            tc.tile_pool(name="pp", bufs=2, space="PSUM") as pp,
            tc.tile_pool(name="gg", bufs=1, space="PSUM") as gg,
        ):
            # ---------------- inputs / constants ----------------
            xp16 = sb.tile([C, PAD * PAD], dt.float16)
            # split the input DMA so conv chunk 0 (rows 0..9) can start early
            nc.sync.dma_start(xp16[:, 0:10 * PAD], xp_d.ap()[:, 0:10 * PAD])
            nc.sync.dma_start(xp16[:, 10 * PAD:], xp_d.ap()[:, 10 * PAD:])
            wt16 = sb.tile([C, 9 * C], dt.float16)
            nc.sync.dma_start(wt16[:], wt_d.ap())

            ident = sb.tile([128, 128], dt.float16)
            make_identity(nc, ident[:])

            out_acc = sb.tile([C, N1], dt.float32)
            # residual init: out_acc = x  (from the padded fp16 input)
            nc.gpsimd.tensor_copy(out_acc[:], _v(xp16[:], PAD + 1, [[PAD, H], [1, W]]))

            f1a = sb.tile([C + 1, N1], dt.float16)
            fT1 = sb.tile([128, 32 * 65], dt.float16)
            nc.vector.memset(_v(fT1[:], C, [[65, 32]]), 1.0)
            nc.vector.memset(f1a[C:C + 1, :], 1.0)

            # ---------------- generic attention (per-superblock emitter) ----------------
            def build_fT(fa, NT, name):
                fT = sb.tile([128, NT * 65], dt.float16, tag=name)
                nc.vector.memset(_v(fT[:], C, [[65, NT]]), 1.0)
                for j in range(NT):
                    pt = pp.tile([128, C], dt.float16, tag="b")
                    nc.tensor.transpose(pt[:], fa[0:C, j * 128:(j + 1) * 128], ident[0:C, 0:C])
                    nc.scalar.copy(fT[:, j * 65:j * 65 + C], pt[:])
                return fT

            def _achunks(N):
                return _chunks(N, 512)

            def attn_A_start(fa, N, isb):
                q0 = isb * 512
                Q = min(512, N - q0)
                nsub = Q // 128
                achunks = _achunks(N)
                multi = len(achunks) > 1
                return dict(
                    fa=fa, N=N, isb=isb, q0=q0, Q=Q, nsub=nsub, achunks=achunks,
                    x1=[cm_pool.tile([128, 8], dt.float32, tag="x1", name=f"x1_{isb}_{s}")
                        for s in range(nsub)] if multi else None,
                    x2=[cm_pool.tile([128, 1], dt.float16, tag="x2", name=f"x2_{isb}_{s}")
                        for s in range(nsub)],
                )

            def attn_A_chunk(st, k):
                fa, q0 = st["fa"], st["q0"]
                off, ln = st["achunks"][k]
                for sub in range(st["nsub"]):
                    lhsA = fa[0:C, q0 + sub * 128: q0 + (sub + 1) * 128]
                    at = aa.tile([128, ln], dt.float32, tag="a")
                    for h0, hl in _chunks(ln, 512):
                        nc.tensor.matmul(at[:, h0:h0 + hl], lhsA,
                                         fa[0:C, off + h0:off + h0 + hl],
                                         start=True, stop=True)
                    if st["x1"] is None:
                        nc.vector.reduce_max(st["x2"][sub][:], at[:], axis=AX.X, negate=True)
                    else:
                        nc.vector.reduce_max(st["x1"][sub][:, k:k + 1], at[:], axis=AX.X)

            def attn_finish(st, fT, write_out, filler=()):
                fa, N, isb = st["fa"], st["N"], st["isb"]
                q0, Q, nsub = st["q0"], st["Q"], st["nsub"]
                NT = N // 128
                nch = len(st["achunks"])
                ga = ga_pool.tile([C + 1, Q], dt.float16, tag="ga")
                nc.vector.tensor_copy(ga[0:C, :], fa[0:C, q0:q0 + Q])
                for sub in range(nsub):
                    x2 = st["x2"][sub]
                    if st["x1"] is not None:
                        nc.vector.reduce_max(x2[:], st["x1"][sub][:, 0:nch],
                                             axis=AX.X, negate=True)
                    # PE-transpose -max [128,1] -> [1,128] into the g_a bias row
                    pt = aa.tile([1, 128], dt.float16, tag="a")
                    nc.tensor.transpose(pt[:], x2[:], ident[:])
                    nc.vector.tensor_copy(ga[C:C + 1, sub * 128:(sub + 1) * 128], pt[:])
                # --- B + exp + C (filler thunks keep PE fed while exp runs) ---
                G = gg.tile([C + 1, Q], dt.float32, tag="g")
                mtiles = list(range(NT))
                groups = [mtiles[i:i + 2] for i in range(0, NT, 2)]
                filler = list(filler)
                fill_at = {int(i * len(groups) / len(filler)): i for i in range(len(filler))} if filler else {}
                for gi, grp in enumerate(groups):
                    if gi in fill_at:
                        filler[fill_at[gi]]()
                    bt = pp.tile([128, 512 * len(grp)], dt.float32, tag="b")
                    et = et_pool.tile([128, 512 * len(grp)], dt.float16, tag="et")
                    for jj, j in enumerate(grp):
                        nc.tensor.matmul(bt[:, jj * 512: jj * 512 + Q],
                                         fa[:, j * 128:(j + 1) * 128], ga[:],
                                         start=True, stop=True)
                    if Q == 512:
                        nc.scalar.activation(et[:], bt[:], AF.Exp)
                    else:
                        for jj in range(len(grp)):
                            nc.scalar.activation(et[:, jj * 512:jj * 512 + Q],
                                                 bt[:, jj * 512:jj * 512 + Q], AF.Exp)
                    for jj, j in enumerate(grp):
                        nc.tensor.matmul(G[:], fT[:, j * 65:(j + 1) * 65],
                                         et[:, jj * 512:jj * 512 + Q],
                                         start=(gi == 0 and jj == 0),
                                         stop=(j == NT - 1))
                # --- D: normalize ---
                Gs = dd_pool.tile([C + 1, 512], dt.float32, tag="gs")
                nc.scalar.copy(Gs[:, 0:Q], G[:])
                linv = dd_pool.tile([1, 512], dt.float32, tag="linv")
                nc.vector.reciprocal(linv[:, 0:Q], Gs[C:C + 1, 0:Q])
                lrep = dd_pool.tile([C, 512], dt.float32, tag="lrep")
                nc.gpsimd.partition_broadcast(lrep[:, 0:Q], linv[0:1, 0:Q])
                write_out(isb, q0, Q, Gs, lrep)

            def w1(isb, q0, Q, Gs, lrep):
                eng = nc.gpsimd
                tmp = dd_pool.tile([C, 512], dt.float32, tag="tmp")
                eng.tensor_tensor(tmp[:, 0:Q], Gs[0:C, 0:Q], lrep[:, 0:Q], op=ALU.mult)
                eng.tensor_tensor(out_acc[:, q0:q0 + Q], out_acc[:, q0:q0 + Q],
                                  tmp[:, 0:Q], op=ALU.add)

            att2p = sb.tile([C, 34 * 34], dt.float32)   # scale-2 attn out, 1-px padded
            att4p = sb.tile([C, 18 * 18], dt.float32)   # scale-4 attn out, 1-px padded
            up_acc = sb.tile([C, N1], dt.float32)       # upsampled x2+x4 sum

            def w2(isb, q0, Q, Gs, lrep):
                r0 = isb * 16
                view = _v(att2p[:], (1 + r0) * 34 + 1, [[34, 16], [1, 32]])
                nc.gpsimd.tensor_tensor(view, Gs[0:C, 0:Q], lrep[:, 0:Q], op=ALU.mult)

            def w4(isb, q0, Q, Gs, lrep):
                view = _v(att4p[:], 18 + 1, [[18, 16], [1, 16]])
                nc.gpsimd.tensor_tensor(view, Gs[0:C, 0:Q], lrep[:, 0:Q], op=ALU.mult)

            # ---------------- pool emitters (gpsimd) ----------------
            f2raw = sb.tile([C, 1024], dt.float32)
            f2a = sb.tile([C + 1, 1024], dt.float16)
            f4a = sb.tile([C + 1, 256], dt.float16)

            def emit_pools2():
                f1 = f1a[0:C, :]
                t2w = sb.tile([C, 2048], dt.float32)
                nc.gpsimd.tensor_tensor(t2w[:], _v(f1, 0, [[2, 2048]]), _v(f1, 1, [[2, 2048]]), op=ALU.add)
                nc.gpsimd.tensor_tensor(f2raw[:], _v(t2w[:], 0, [[64, 32], [1, 32]]),
                                        _v(t2w[:], 32, [[64, 32], [1, 32]]), op=ALU.add)
                nc.gpsimd.tensor_scalar_mul(f2a[0:C, :], f2raw[:], 0.25)
                nc.gpsimd.memset(f2a[C:C + 1, :], 1.0)

            def emit_pools4():
                t4w = sb.tile([C, 512], dt.float32)
                nc.gpsimd.tensor_tensor(t4w[:], _v(f2raw[:], 0, [[2, 512]]), _v(f2raw[:], 1, [[2, 512]]), op=ALU.add)
                f4raw = sb.tile([C, 256], dt.float32)
                nc.gpsimd.tensor_tensor(f4raw[:], _v(t4w[:], 0, [[32, 16], [1, 16]]),
                                        _v(t4w[:], 16, [[32, 16], [1, 16]]), op=ALU.add)
                nc.gpsimd.tensor_scalar_mul(f4a[0:C, :], f4raw[:], 1.0 / 16.0)
                nc.gpsimd.memset(f4a[C:C + 1, :], 1.0)

            # ---------------- upsample emitters ----------------
            def emit_up4():
                p4 = att4p[:]
                ups = sb.tile([C, 256], dt.float32, tag="ups4")
                # edge replication (cols then rows so corners fill correctly)
                nc.gpsimd.tensor_copy(_v(p4, 18, [[18, 16]]), _v(p4, 19, [[18, 16]]))
                nc.gpsimd.tensor_copy(_v(p4, 18 + 17, [[18, 16]]), _v(p4, 18 + 16, [[18, 16]]))
                nc.gpsimd.tensor_copy(_v(p4, 0, [[1, 18]]), _v(p4, 18, [[1, 18]]))
                nc.gpsimd.tensor_copy(_v(p4, 17 * 18, [[1, 18]]), _v(p4, 16 * 18, [[1, 18]]))
                # W-stage: t4u rows 1..16 (padded layout [C, 18, 64]) on gpsimd
                t4u = sb.tile([C, 18 * 64], dt.float32)
                pre58 = sb.tile([C, 256], dt.float32)   # 0.625 * center
                pre78 = sb.tile([C, 256], dt.float32)   # 0.875 * center
                ctr = _v(p4, 18 + 1, [[18, 16], [1, 16]])
                nc.gpsimd.tensor_scalar_mul(pre58[:], ctr, 0.625)
                nc.gpsimd.tensor_scalar_mul(pre78[:], ctr, 0.875)
                lft = _v(p4, 18 + 0, [[18, 16], [1, 16]])
                rgt = _v(p4, 18 + 2, [[18, 16], [1, 16]])
                for p, (nb, a, pre) in enumerate([(lft, 0.375, pre58), (lft, 0.125, pre78),
                                                  (rgt, 0.125, pre78), (rgt, 0.375, pre58)]):
                    outv = _v(t4u[:], 64 + p, [[64, 16], [4, 16]])
                    nc.gpsimd.tensor_scalar_mul(ups[:], nb, a)
                    nc.gpsimd.tensor_tensor(outv, ups[:], pre[:], op=ALU.add)
                nc.gpsimd.tensor_copy(_v(t4u[:], 0, [[1, 64]]), _v(t4u[:], 64, [[1, 64]]))
                nc.gpsimd.tensor_copy(_v(t4u[:], 17 * 64, [[1, 64]]), _v(t4u[:], 16 * 64, [[1, 64]]))
                # H-stage into up_acc (rows I = 4r+p): first op writes, second accumulates
                u4s = sb.tile([C, 1024], dt.float32)
                for p, (o1, a1, o2, a2) in enumerate([(0, 0.375, 64, 0.625), (0, 0.125, 64, 0.875),
                                                      (64, 0.875, 128, 0.125), (64, 0.625, 128, 0.375)]):
                    outv = _v(up_acc[:], p * 64, [[256, 16], [1, 64]])
                    nc.gpsimd.tensor_scalar_mul(outv, _v(t4u[:], o1, [[64, 16], [1, 64]]), a1)
                    nc.gpsimd.tensor_scalar_mul(u4s[:], _v(t4u[:], o2, [[64, 16], [1, 64]]), a2)
                    nc.gpsimd.tensor_tensor(outv, outv, u4s[:], op=ALU.add)

            def emit_up2():
                p2 = att2p[:]
                ups = sb.tile([C, 1024], dt.float32, tag="ups2")
                nc.gpsimd.tensor_copy(_v(p2, 34, [[34, 32]]), _v(p2, 35, [[34, 32]]))
                nc.gpsimd.tensor_copy(_v(p2, 34 + 33, [[34, 32]]), _v(p2, 34 + 32, [[34, 32]]))
                nc.gpsimd.tensor_copy(_v(p2, 0, [[1, 34]]), _v(p2, 34, [[1, 34]]))
                nc.gpsimd.tensor_copy(_v(p2, 33 * 34, [[1, 34]]), _v(p2, 32 * 34, [[1, 34]]))
                t2u = sb.tile([C, 34 * 64], dt.float32)
                pre34 = sb.tile([C, 1024], dt.float32)  # 0.75 * center
                ctr2 = _v(p2, 34 + 1, [[34, 32], [1, 32]])
                nc.gpsimd.tensor_scalar_mul(pre34[:], ctr2, 0.75)
                lft2 = _v(p2, 34 + 0, [[34, 32], [1, 32]])
                rgt2 = _v(p2, 34 + 2, [[34, 32], [1, 32]])
                for p, nb in enumerate([lft2, rgt2]):
                    outv = _v(t2u[:], 64 + p, [[64, 32], [2, 32]])
                    nc.gpsimd.tensor_scalar_mul(ups[:], nb, 0.25)
                    nc.gpsimd.tensor_tensor(outv, ups[:], pre34[:], op=ALU.add)
                nc.gpsimd.tensor_copy(_v(t2u[:], 0, [[1, 64]]), _v(t2u[:], 64, [[1, 64]]))
                nc.gpsimd.tensor_copy(_v(t2u[:], 33 * 64, [[1, 64]]), _v(t2u[:], 32 * 64, [[1, 64]]))
                u2s = sb.tile([C, 2048], dt.float32)
                for p, (o1, a1, o2, a2) in enumerate([(0, 0.25, 64, 0.75), (64, 0.75, 128, 0.25)]):
                    outv = _v(up_acc[:], p * 64, [[128, 32], [1, 64]])
                    for off, coef in ((o1, a1), (o2, a2)):
                        nc.gpsimd.tensor_scalar_mul(u2s[:], _v(t2u[:], off, [[64, 32], [1, 64]]), coef)
                        nc.gpsimd.tensor_tensor(outv, outv, u2s[:], op=ALU.add)

            def attn_sb(fa, fT, N, isb, write_out):
                st = attn_A_start(fa, N, isb)
                for k in range(len(st["achunks"])):
                    attn_A_chunk(st, k)
                attn_finish(st, fT, write_out)

            # ---------------- master schedule ----------------
            # conv chunks interleaved with fT1 build and sb0/sb1's A-pass
            # (A-chunk k only needs conv chunk k evicted).
            st0 = attn_A_start(f1a, N1, 0)
            st1 = attn_A_start(f1a, N1, 1)
            # A-chunk (st, k) becomes runnable once conv has evicted its columns
            asched = {0: [(st0, 0)], 1: [(st1, 0), (st0, 1)], 2: [(st1, 1), (st0, 2)],
                      3: [(st1, 2), (st0, 3)], 4: [(st1, 3), (st0, 4)],
                      5: [(st1, 4), (st0, 5)], 6: [(st1, 5), (st0, 6)],
                      7: [(st1, 6), (st0, 7)]}
            for r in range(8):
                cp = pp.tile([C, 512], dt.float32, tag="b")
                for tap in range(9):
                    dy, dx = divmod(tap, 3)
                    rhs = _v(xp16[:], (8 * r + dy) * PAD + dx, [[PAD, 8], [1, W]])
                    nc.tensor.matmul(cp[:], wt16[:, tap * C:(tap + 1) * C], rhs,
                                     start=(tap == 0), stop=(tap == 8))
                nc.scalar.copy(f1a[0:C, r * 512:(r + 1) * 512], cp[:])
                for st, k in asched.get(r, []):
                    attn_A_chunk(st, k)
                for j in range(4 * r, 4 * r + 4):
                    pt = pp.tile([128, C], dt.float16, tag="b")
                    nc.tensor.transpose(pt[:], f1a[0:C, j * 128:(j + 1) * 128], ident[0:C, 0:C])
                    nc.scalar.copy(fT1[:, j * 65:j * 65 + C], pt[:])
            attn_A_chunk(st1, 7)

            def fill_chunks(st):
                return [(lambda st=st, k=k: attn_A_chunk(st, k))
                        for k in range(len(st["achunks"]))]

            emit_pools2()
            st2 = attn_A_start(f1a, N1, 2)
            attn_finish(st0, fT1, w1, filler=fill_chunks(st2))
            st3 = attn_A_start(f1a, N1, 3)
            attn_finish(st1, fT1, w1, filler=fill_chunks(st3))
            fT2 = build_fT(f2a, 8, "fT2")
            st4 = attn_A_start(f1a, N1, 4)
            attn_finish(st2, fT1, w1, filler=fill_chunks(st4))
            attn_sb(f2a, fT2, 1024, 0, w2)
            st5 = attn_A_start(f1a, N1, 5)
            attn_finish(st3, fT1, w1, filler=fill_chunks(st5))
            attn_sb(f2a, fT2, 1024, 1, w2)
            emit_pools4()
            st6 = attn_A_start(f1a, N1, 6)
            attn_finish(st4, fT1, w1, filler=fill_chunks(st6))
            fT4 = build_fT(f4a, 2, "fT4")
            attn_sb(f4a, fT4, 256, 0, w4)
            emit_up4()
            st7 = attn_A_start(f1a, N1, 7)
            attn_finish(st5, fT1, w1, filler=fill_chunks(st7))
            emit_up2()
            attn_finish(st6, fT1, w1)
            # last superblock: the final up_acc add + most of the output DMA
            # overlap its B/C window (DVE/DMA are otherwise idle there).
            nc.vector.tensor_tensor(out_acc[:, 0:3584], out_acc[:, 0:3584],
                                    up_acc[:, 0:3584], op=ALU.add)
            nc.sync.dma_start(out_d.ap()[:, 0:3584], out_acc[:, 0:3584])
            attn_finish(st7, fT1, w1)
            nc.gpsimd.tensor_tensor(out_acc[:, 3584:N1], out_acc[:, 3584:N1],
                                    up_acc[:, 3584:N1], op=ALU.add)
            nc.sync.dma_start(out_d.ap()[:, 3584:N1], out_acc[:, 3584:N1])

    nc.compile()
    return nc


def _prep_inputs(x, W_std):
    lap = np.array([[0., 1., 0.], [1., -4., 1.], [0., 1., 0.]], dtype=np.float32)
    Wl = W_std.astype(np.float32) + lap[None, None] * np.eye(C, dtype=np.float32)[:, :, None, None]
    wt = np.ascontiguousarray(Wl.transpose(1, 2, 3, 0).reshape(C, 9 * C)).astype(np.float16)
    B = x.shape[0]
    xps = np.zeros((B, C, PAD, PAD), dtype=np.float16)
    xps[:, :, 1:H + 1, 1:W + 1] = x.astype(np.float16)
    return xps.reshape(B, C, PAD * PAD), wt


def _run(x, W_std, trace=False):
    x = np.asarray(x)
    W_std = np.asarray(W_std)
    xps, wt = _prep_inputs(x, W_std)
    if "nc" not in _cache:
        _cache["nc"] = _build_nc()
    nc = _cache["nc"]
    in_maps = [{"xp": np.ascontiguousarray(xps[i]), "wt": wt} for i in range(x.shape[0])]
    ncores = min(NCORES, x.shape[0])
    res = run_bass_kernel_spmd(nc, in_maps, core_ids=list(range(ncores)), trace=trace)
    out = np.stack([res.results[i]["out"].reshape(C, H, W) for i in range(x.shape[0])])
    return out.astype(np.float32), res


def kernel(x, W_std):
    out, _ = _run(x, W_std, trace=False)
    return out



# revision 34
# speedup vs baseline: 1.4257x; 1.4257x over previous
"""Trainium2 Bass kernel for nn_BoundaryExtractionModule.

Data-parallel over batch: 8 samples -> 8 NeuronCores, one sample per core.

Per-core pipeline (channel-major [C, N], C=64, all SBUF data bf16):
  conv3x3(W_std)+depthwise-Laplacian : tap-PAIRED shift-matmuls (6 per 512-col
      block instead of 9) using a host-built column-shifted second copy of the
      input on partitions 64..127 (contraction K=128 packs 2 taps).
  3-scale non-local attention WITHOUT a row-max pass:
      The softmax bias B_q only needs to be within about (-76, +40) nats of the
      true row max when E is stored in bf16 (8-bit exponent): entries that
      overflow the window are clamped at 1e30 (their ratios flatten), entries
      below it vanish harmlessly.  We use B_q = (max of S over the DIAGONAL
      512-column block) + 60, computed from the 4 diagonal-block S tiles with
      Pool-engine partition-axis max-reduces.  Scales 2/4 just use the
      diagonal ||f_q||^2 (exact row max there, measured).
      B-pass (S^T with bias folded via augmented K=65 contraction row) ->
      exp on Act (PSUM fp32 -> bf16 SBUF) -> clamp on DVE (4x mode) ->
      PV matmul with ones-column for the denominator.
  2x2 poolings as PE matmuls against a constant pooling matrix (host const).
  f^T tiles (PV lhsT) via DMA-engine transposes (free engines).
  bilinear x2/x4 upsample on DVE in bf16; residual add; bf16 output DMA
  (host upcasts to fp32).
"""

import numpy as np

import concourse.bass as bass
import concourse.mybir as mybir
import concourse.tile as tile
from concourse import bacc
from concourse.bass_utils import run_bass_kernel_spmd
from concourse.masks import make_identity

dt = mybir.dt
AF = mybir.ActivationFunctionType
ALU = mybir.AluOpType
AX = mybir.AxisListType

C = 64
H = W = 64
N1 = H * W          # 4096
N2 = 1024
N4 = 256
PAD = 66
NCORES = 8
FT = 80          # fT tile stride (multiple of 16 for XBAR dst alignment)
BOFF = 60.0         # bias offset (nats) delaying the clamp window
CLAMP = 1.0e30

_cache = {}


def _v(ap, off, dims):
    """View of `ap` at free-offset `off` with free dims `dims` (keeps partition dim)."""
    return bass.AP(ap.tensor, ap.offset + off, [list(ap.ap[0])] + [list(d) for d in dims])


def _build_nc(debug=False):
    nc = bacc.Bacc(None, target_bir_lowering=False)
    xp_d = nc.dram_tensor("xp", [128, PAD * PAD], dt.bfloat16, kind="ExternalInput")
    wt_d = nc.dram_tensor("wt", [128, 6 * C], dt.bfloat16, kind="ExternalInput")
    pp_d = nc.dram_tensor("pp", [128, 64], dt.bfloat16, kind="ExternalInput")
    out_d = nc.dram_tensor("out", [C, N1], dt.bfloat16, kind="ExternalOutput")
    if debug:
        dbg = {n: nc.dram_tensor(n, shape, dtp, kind="ExternalOutput")
               for n, shape, dtp in [
                   ("d_f1a", [C + 1, N1], dt.bfloat16),
                   ("d_fT1", [128, 32 * FT], dt.bfloat16),
                   ("d_f2a", [C + 1, N2], dt.bfloat16),
                   ("d_fT2", [128, 8 * FT], dt.bfloat16),
                   ("d_f4a", [C + 1, N4], dt.bfloat16),
                   ("d_bias1", [1, N1], dt.bfloat16),
                   ("d_bias2", [1, N2], dt.bfloat16),
                   ("d_att2", [C, 34 * 34], dt.bfloat16),
                   ("d_att4", [C, 18 * 18], dt.bfloat16),
                   ("d_upacc", [C, N1], dt.bfloat16),
               ]}

    with tile.TileContext(nc) as tc:
        with (
            tc.tile_pool(name="sb", bufs=1) as sb,
            tc.tile_pool(name="ga", bufs=11) as ga_pool,
            tc.tile_pool(name="et", bufs=6) as et_pool,
            tc.tile_pool(name="dd", bufs=6) as dd_pool,
            tc.tile_pool(name="bm", bufs=8) as bm_pool,
            tc.tile_pool(name="bt", bufs=3, space="PSUM") as bt_pool,
            tc.tile_pool(name="gg", bufs=2, space="PSUM") as gg_pool,
        ):
            # ---------------- inputs / constants ----------------
            xp2 = sb.tile([128, PAD * PAD], dt.bfloat16)
            wt = sb.tile([128, 6 * C], dt.bfloat16)
            # chunked input loads so conv block 0 can start early
            nc.sync.dma_start(wt[:, 0:3 * C], wt_d.ap()[:, 0:3 * C])
            nc.sync.dma_start(xp2[:, 0:10 * PAD], xp_d.ap()[:, 0:10 * PAD])
            nc.sync.dma_start(wt[:, 3 * C:], wt_d.ap()[:, 3 * C:])
            for r in range(1, 8):
                lo, hi = (8 * r + 2) * PAD, min(PAD, 8 * r + 10) * PAD
                nc.sync.dma_start(xp2[:, lo:hi], xp_d.ap()[:, lo:hi])
            pmat = sb.tile([128, 64], dt.bfloat16)
            nc.sync.dma_start(pmat[:], pp_d.ap())

            ident = sb.tile([128, 128], dt.bfloat16)
            make_identity(nc, ident[:])
            negones = sb.tile([C, 1], dt.bfloat16)
            nc.vector.memset(negones[:], -1.0)

            # persistent feature tensors
            f1a = sb.tile([C + 1, N1], dt.bfloat16)
            nc.vector.memset(f1a[C:C + 1, :], 1.0)
            fT1 = sb.tile([128, 32 * FT], dt.bfloat16)
            nc.vector.memset(_v(fT1[:], C, [[FT, 32]]), 1.0)
            f2a = sb.tile([C + 1, N2], dt.bfloat16)
            nc.vector.memset(f2a[C:C + 1, :], 1.0)
            fT2 = sb.tile([128, 8 * FT], dt.bfloat16)
            nc.vector.memset(_v(fT2[:], C, [[FT, 8]]), 1.0)
            f4a = sb.tile([C + 1, N4], dt.bfloat16)
            nc.vector.memset(f4a[C:C + 1, :], 1.0)
            fT4 = sb.tile([128, 2 * FT], dt.bfloat16)
            nc.vector.memset(_v(fT4[:], C, [[FT, 2]]), 1.0)

            out_acc = sb.tile([C, N1], dt.bfloat16)
            up_acc = sb.tile([C, N1], dt.bfloat16)
            att2p = sb.tile([C, 34 * 34], dt.bfloat16)
            att4p = sb.tile([C, 18 * 18], dt.bfloat16)



            # ---------------- generic attention emitters ----------------
            def emit_diag_bias_q(fa, isb, name):
                """Startup variant: q-major diagonal block, DVE free-axis reduce."""
                q0 = isb * 512
                bias = bm_pool.tile([1, 512], dt.bfloat16, tag="bias", name=f"bias_{name}_{isb}")
                xms = []
                for sub in range(2):
                    at = bt_pool.tile([128, 1024], dt.float32, tag="bt", name=f"atq_{name}_{isb}_{sub}")
                    for h in range(2):
                        nc.tensor.matmul(at[:, h * 512:(h + 1) * 512],
                                         fa[0:C, q0 + (2 * sub + h) * 128: q0 + (2 * sub + h + 1) * 128],
                                         fa[0:C, q0:q0 + 512], start=True, stop=True)
                    for h in range(2):
                        xm = bm_pool.tile([128, 1], dt.bfloat16, tag="xm",
                                          name=f"xm_{name}_{isb}_{2 * sub + h}")
                        nc.vector.reduce_max(xm[:], at[:, h * 512:(h + 1) * 512],
                                             axis=AX.X, negate=True)
                        xms.append(xm)
                def finish():
                    ptt = bt_pool.tile([128, 1024], dt.bfloat16, tag="bt", name=f"ptt_{name}_{isb}")
                    for sub in range(4):
                        nc.tensor.transpose(ptt[0:1, sub * 128:(sub + 1) * 128], xms[sub][:], ident[:])
                    nc.vector.tensor_scalar(bias[:], ptt[0:1, 0:512], 1.0, -BOFF,
                                            op0=ALU.mult, op1=ALU.add)
                    if debug:
                        nc.sync.dma_start(dbg["d_bias1"].ap()[:, isb * 512:(isb + 1) * 512], bias[:])
                return bias, finish

            def emit_ga(fa, isb, bias, Q=512):
                ga = ga_pool.tile([C + 1, 512], dt.bfloat16, tag="ga")
                nc.vector.tensor_copy(ga[0:C, 0:Q], fa[0:C, isb * 512: isb * 512 + Q])
                nc.vector.tensor_copy(ga[C:C + 1, 0:Q], bias[:, 0:Q])
                return ga

            def emit_group(fa, fT, ga, NT, jj0, Q=512):
                """One B->exp->clamp->PV group covering m-tiles jj0, jj0+1."""
                njj = min(2, NT - jj0)
                bt = bt_pool.tile([128, 1024], dt.float32, tag="bt")
                for i in range(njj):
                    j = jj0 + i
                    nc.tensor.matmul(bt[:, i * 512: i * 512 + Q],
                                     fa[:, j * 128:(j + 1) * 128], ga[0:C + 1, 0:Q],
                                     start=True, stop=True)
                et = et_pool.tile([128, 1024], dt.bfloat16, tag="et")
                if njj == 2 and Q == 512:
                    nc.scalar.activation(et[:], bt[:], AF.Exp)
                    nc.vector.tensor_scalar_min(et[:], et[:], CLAMP)
                else:
                    for i in range(njj):
                        nc.scalar.activation(et[:, i * 512: i * 512 + Q],
                                             bt[:, i * 512: i * 512 + Q], AF.Exp)
                        nc.vector.tensor_scalar_min(et[:, i * 512: i * 512 + Q],
                                                    et[:, i * 512: i * 512 + Q], CLAMP)
                return et, njj

            def emit_pv(fT, G, et, NT, jj0, Q=512):
                njj = min(2, NT - jj0)
                for i in range(njj):
                    j = jj0 + i
                    nc.tensor.matmul(G[:, 0:Q], fT[:, j * FT: j * FT + 65],
                                     et[:, i * 512: i * 512 + Q],
                                     start=(j == 0), stop=(j == NT - 1))

            def emit_norm(G, write_out, isb, Q=512):
                Gs = dd_pool.tile([C + 1, 512], dt.bfloat16, tag="gs")
                nc.vector.tensor_copy(Gs[:, 0:Q], G[:, 0:Q])
                linv = dd_pool.tile([1, 512], dt.bfloat16, tag="linv")
                with nc.allow_low_precision("softmax denominators are bf16-accurate here"):
                    nc.vector.reciprocal(linv[:, 0:Q], Gs[C:C + 1, 0:Q])
                lrep = dd_pool.tile([C, 512], dt.bfloat16, tag="lrep")
                nc.gpsimd.partition_broadcast(lrep[:, 0:Q], linv[0:1, 0:Q])
                write_out(isb, Q, Gs, lrep)

            def w1(isb, Q, Gs, lrep):
                q0 = isb * 512
                tmp = dd_pool.tile([C, 512], dt.bfloat16, tag="tmp")
                nc.vector.tensor_tensor(tmp[:, 0:Q], Gs[0:C, 0:Q], lrep[:, 0:Q], op=ALU.mult)
                xview = _v(xp2[0:C, :], (8 * isb + 1) * PAD + 1, [[PAD, 8], [1, W]])
                nc.vector.tensor_tensor(out_acc[:, q0:q0 + Q], xview, tmp[:, 0:Q], op=ALU.add)

            def w2(isb, Q, Gs, lrep):
                r0 = isb * 16
                view = _v(att2p[:], (1 + r0) * 34 + 1, [[34, 16], [1, 32]])
                nc.vector.tensor_tensor(view, Gs[0:C, 0:Q], lrep[:, 0:Q], op=ALU.mult)

            def w4(isb, Q, Gs, lrep):
                view = _v(att4p[:], 18 + 1, [[18, 16], [1, 16]])
                nc.vector.tensor_tensor(view, Gs[0:C, 0:Q], lrep[:, 0:Q], op=ALU.mult)

            # ---------------- conv + fT1 + pooled fT2 (wavefront) ----------------
            # conv block r (8 image rows): 3 paired + 3 single tap matmuls
            def emit_conv_block(r):
                cp = bt_pool.tile([128, 1024], dt.float32, tag="bt", name=f"cp_{r}")
                for dy in range(3):
                    rhs2 = _v(xp2[:], (8 * r + dy) * PAD + 0, [[PAD, 8], [1, W]])
                    nc.tensor.matmul(cp[0:C, 0:512], wt[:, dy * C:(dy + 1) * C], rhs2,
                                     start=(dy == 0), stop=False)
                for dy in range(3):
                    rhs1 = _v(xp2[0:C, :], (8 * r + dy) * PAD + 2, [[PAD, 8], [1, W]])
                    nc.tensor.matmul(cp[0:C, 0:512], wt[0:C, (3 + dy) * C:(4 + dy) * C], rhs1,
                                     start=False, stop=(dy == 2))
                fstg = dd_pool.tile([C, 512], dt.bfloat16, tag="fstg", bufs=3, name=f"fstg_{r}")
                nc.vector.tensor_copy(fstg[:], cp[0:C, 0:512])
                nc.vector.tensor_copy(f1a[0:C, r * 512:(r + 1) * 512], fstg[:])
                return fstg

            def emit_ft1_block(r, fstg):
                # batched XBAR transpose: out[p, b*65+c] = in[c, b*128+p]
                # (source must be a contiguous tile; strided views mis-lower)
                nc.sync.dma_start_transpose(
                    _v(fT1[:], 4 * r * FT, [[FT, 4], [1, C]]), fstg[:])

            def emit_pool2_block(r):
                pt = bt_pool.tile([128, 1024], dt.float32, tag="bt", name=f"p2_{r}")
                for k in range(4):
                    j = 4 * r + k
                    nc.tensor.matmul(pt[0:32, k * C:(k + 1) * C], pmat[:, 0:32],
                                     fT1[:, j * FT: j * FT + C], start=True, stop=True)
                for k in range(4):
                    nc.vector.tensor_copy(fT2[32 * k:32 * (k + 1), r * FT: r * FT + C],
                                          pt[0:32, k * C:(k + 1) * C])

            # ---------------- scale-2 / scale-4 prep ----------------
            def emit_f2a():
                for g in range(8):
                    pt = bt_pool.tile([128, 1024], dt.bfloat16, tag="bt", name=f"t2_{g}")
                    nc.tensor.transpose(pt[0:C, 0:128], fT2[:, g * FT: g * FT + C], ident[:])
                    nc.vector.tensor_copy(f2a[0:C, g * 128:(g + 1) * 128], pt[0:C, 0:128])

            def emit_f4():
                for g2 in range(2):
                    pt = bt_pool.tile([128, 1024], dt.float32, tag="bt", name=f"p4_{g2}")
                    for k in range(4):
                        g = 4 * g2 + k
                        nc.tensor.matmul(pt[0:32, k * C:(k + 1) * C], pmat[:, 32:64],
                                         fT2[:, g * FT: g * FT + C], start=True, stop=True)
                    for k in range(4):
                        nc.vector.tensor_copy(fT4[32 * k:32 * (k + 1), g2 * FT: g2 * FT + C],
                                              pt[0:32, k * C:(k + 1) * C])
                for g2 in range(2):
                    pt = bt_pool.tile([128, 1024], dt.bfloat16, tag="bt", name=f"t4_{g2}")
                    nc.tensor.transpose(pt[0:C, 0:128], fT4[:, g2 * FT: g2 * FT + C], ident[:])
                    nc.vector.tensor_copy(f4a[0:C, g2 * 128:(g2 + 1) * 128], pt[0:C, 0:128])

            def emit_diag_bias_sq(fa, N, name):
                """bias rows from exact diagonal: -(sum_c f^2) - BOFF, via (-1)s matmul."""
                sq = dd_pool.tile([C, N], dt.bfloat16, tag=f"sq_{name}")
                nc.vector.tensor_tensor(sq[:], fa[0:C, 0:N], fa[0:C, 0:N], op=ALU.mult)
                biases = []
                for k in range((N + 511) // 512):
                    Q = min(512, N - k * 512)
                    nd = bt_pool.tile([128, 1024], dt.float32, tag="bt", name=f"nd_{name}_{k}")
                    nc.tensor.matmul(nd[0:1, 0:Q], negones[:], sq[:, k * 512: k * 512 + Q],
                                     start=True, stop=True)
                    bias = bm_pool.tile([1, 512], dt.bfloat16, tag="bias", name=f"bias_{name}_{k}")
                    nc.vector.tensor_scalar(bias[:, 0:Q], nd[0:1, 0:Q], 1.0, -BOFF,
                                            op0=ALU.mult, op1=ALU.add)
                    if debug and name == "s2":
                        nc.sync.dma_start(dbg["d_bias2"].ap()[:, k * 512:(k + 1) * 512], bias[:, 0:Q])
                    biases.append(bias)
                return biases

            # ---------------- upsample emitters (bf16, DVE), chunked ----------------
            def up4_chunks():
                p4 = att4p[:]
                ups = sb.tile([C, 256], dt.bfloat16, tag="ups4", name="ups4")
                t4u = sb.tile([C, 18 * 64], dt.bfloat16, name="t4u")
                pre58 = sb.tile([C, 256], dt.bfloat16, name="pre58")
                pre78 = sb.tile([C, 256], dt.bfloat16, name="pre78")
                u4s = sb.tile([C, 1024], dt.bfloat16, name="u4s")
                ctr = _v(p4, 18 + 1, [[18, 16], [1, 16]])
                lft = _v(p4, 18 + 0, [[18, 16], [1, 16]])
                rgt = _v(p4, 18 + 2, [[18, 16], [1, 16]])

                def c0():
                    nc.vector.tensor_copy(_v(p4, 18, [[18, 16]]), _v(p4, 19, [[18, 16]]))
                    nc.vector.tensor_copy(_v(p4, 18 + 17, [[18, 16]]), _v(p4, 18 + 16, [[18, 16]]))
                    nc.vector.tensor_copy(_v(p4, 0, [[1, 18]]), _v(p4, 18, [[1, 18]]))
                    nc.vector.tensor_copy(_v(p4, 17 * 18, [[1, 18]]), _v(p4, 16 * 18, [[1, 18]]))
                    nc.vector.tensor_scalar_mul(pre58[:], ctr, 0.625)
                    nc.vector.tensor_scalar_mul(pre78[:], ctr, 0.875)

                def cw(p, nb, a, pre):
                    def f():
                        outv = _v(t4u[:], 64 + p, [[64, 16], [4, 16]])
                        nc.vector.tensor_scalar_mul(ups[:], nb, a)
                        nc.vector.tensor_tensor(outv, ups[:], pre[:], op=ALU.add)
                    return f

                def cmid():
                    nc.vector.tensor_copy(_v(t4u[:], 0, [[1, 64]]), _v(t4u[:], 64, [[1, 64]]))
                    nc.vector.tensor_copy(_v(t4u[:], 17 * 64, [[1, 64]]), _v(t4u[:], 16 * 64, [[1, 64]]))

                def ch(p, o1, a1, o2, a2):
                    def f():
                        outv = _v(up_acc[:], p * 64, [[256, 16], [1, 64]])
                        nc.vector.tensor_scalar_mul(outv, _v(t4u[:], o1, [[64, 16], [1, 64]]), a1)
                        nc.vector.tensor_scalar_mul(u4s[:], _v(t4u[:], o2, [[64, 16], [1, 64]]), a2)
                        nc.vector.tensor_tensor(outv, outv, u4s[:], op=ALU.add)
                    return f

                return [c0,
                        cw(0, lft, 0.375, pre58), cw(1, lft, 0.125, pre78),
                        cw(2, rgt, 0.125, pre78), cw(3, rgt, 0.375, pre58),
                        cmid,
                        ch(0, 0, 0.375, 64, 0.625), ch(1, 0, 0.125, 64, 0.875),
                        ch(2, 64, 0.875, 128, 0.125), ch(3, 64, 0.625, 128, 0.375)]

            def up2_chunks():
                p2 = att2p[:]
                ups = sb.tile([C, 1024], dt.bfloat16, tag="ups2", name="ups2")
                t2u = sb.tile([C, 34 * 64], dt.bfloat16, name="t2u")
                pre34 = sb.tile([C, 1024], dt.bfloat16, name="pre34")
                u2s = sb.tile([C, 2048], dt.bfloat16, name="u2s")
                ctr2 = _v(p2, 34 + 1, [[34, 32], [1, 32]])
                lft2 = _v(p2, 34 + 0, [[34, 32], [1, 32]])
                rgt2 = _v(p2, 34 + 2, [[34, 32], [1, 32]])

                def c0():
                    nc.vector.tensor_copy(_v(p2, 34, [[34, 32]]), _v(p2, 35, [[34, 32]]))
                    nc.vector.tensor_copy(_v(p2, 34 + 33, [[34, 32]]), _v(p2, 34 + 32, [[34, 32]]))
                    nc.vector.tensor_copy(_v(p2, 0, [[1, 34]]), _v(p2, 34, [[1, 34]]))
                    nc.vector.tensor_copy(_v(p2, 33 * 34, [[1, 34]]), _v(p2, 32 * 34, [[1, 34]]))
                    nc.vector.tensor_scalar_mul(pre34[:], ctr2, 0.75)

                def cw(p, nb):
                    def f():
                        outv = _v(t2u[:], 64 + p, [[64, 32], [2, 32]])
                        nc.vector.tensor_scalar_mul(ups[:], nb, 0.25)
                        nc.vector.tensor_tensor(outv, ups[:], pre34[:], op=ALU.add)
                    return f

                def cmid():
                    nc.vector.tensor_copy(_v(t2u[:], 0, [[1, 64]]), _v(t2u[:], 64, [[1, 64]]))
                    nc.vector.tensor_copy(_v(t2u[:], 33 * 64, [[1, 64]]), _v(t2u[:], 32 * 64, [[1, 64]]))

                def ch(p, off, coef):
                    def f():
                        outv = _v(up_acc[:], p * 64, [[128, 32], [1, 64]])
                        nc.vector.tensor_scalar_mul(u2s[:], _v(t2u[:], off, [[64, 32], [1, 64]]), coef)
                        nc.vector.tensor_tensor(outv, outv, u2s[:], op=ALU.add)
                    return f

                return [c0, cw(0, lft2), cw(1, rgt2), cmid,
                        ch(0, 0, 0.25), ch(0, 64, 0.75), ch(1, 64, 0.75), ch(1, 128, 0.25)]

            # ---------------- master schedule ----------------
            # conv wavefront: after conv block r, emit diagA(r); from r>=1 also
            # stream the first superblock's B-groups whose m-tiles are ready.
            biases1 = [None] * 8
            ga1 = [None] * 8
            G1 = [None] * 8

            def sb1_group(isb, g):
                if G1[isb] is None:
                    G1[isb] = gg_pool.tile([C + 1, 512], dt.float32, tag="g", name=f"G1_{isb}")
                et, _ = emit_group(f1a, fT1, ga1[isb], 32, 2 * g)
                emit_pv(fT1, G1[isb], et, 32, 2 * g)

            pending_dq = None
            for r in range(8):
                fstg = emit_conv_block(r)
                emit_ft1_block(r, fstg)
                if pending_dq is not None:
                    k, fin = pending_dq
                    fin()
                    ga1[k] = emit_ga(f1a, k, biases1[k])
                    pending_dq = None
                if r <= 1:
                    biases1[r], dq_fin = emit_diag_bias_q(f1a, r, "s1")
                    pending_dq = (r, dq_fin)
                if r >= 1:
                    emit_pool2_block(r - 1)
                    sb1_group(0, 2 * (r - 1))
                    sb1_group(0, 2 * (r - 1) + 1)
            pending_fin = [None]

            def prep_sb(k):
                if k < 8 and biases1[k] is None:
                    biases1[k], fin = emit_diag_bias_q(f1a, k, "s1")
                    pending_fin[0] = (k, fin)

            def prep_sb_fin():
                if pending_fin[0] is not None:
                    k, fin = pending_fin[0]
                    fin()
                    ga1[k] = emit_ga(f1a, k, biases1[k])
                    pending_fin[0] = None

            prep_hooks = {}

            def sb1_part(isb, g0, g1):
                for g in range(g0, g1):
                    sb1_group(isb, g)
                    if g == 8:
                        prep_sb(isb + 1)
                    elif g == 11:
                        prep_sb_fin()
                    hook = prep_hooks.get((isb, g))
                    if hook is not None:
                        hook()

            def warm_sb(isb):
                """B+exp+clamp of groups 0,1 (no PV — G ring slot not free yet)."""
                ets = []
                for g in (0, 1):
                    et, _ = emit_group(f1a, fT1, ga1[isb], 32, 2 * g)
                    ets.append((g, et))
                return ets

            def finish_warm(isb, ets):
                if G1[isb] is None:
                    G1[isb] = gg_pool.tile([C + 1, 512], dt.float32, tag="g", name=f"G1_{isb}")
                for g, et in ets:
                    emit_pv(fT1, G1[isb], et, 32, 2 * g)

            b2 = [None, None]
            b4 = [None]
            prep_hooks[(1, 2)] = emit_f2a
            prep_hooks[(1, 4)] = emit_f4
            prep_hooks[(1, 11)] = lambda: b2.__setitem__(slice(None), emit_diag_bias_sq(f2a, N2, "s2"))
            prep_hooks[(1, 13)] = lambda: b4.__setitem__(slice(None), emit_diag_bias_sq(f4a, N4, "s4"))

            emit_pool2_block(7)
            sb1_group(0, 14)
            sb1_group(0, 15)
            w_next = warm_sb(1)
            emit_norm(G1[0], w1, 0)

            def s2_sb(isb):
                ga = emit_ga(f2a, isb, b2[isb])
                G = gg_pool.tile([C + 1, 512], dt.float32, tag="g", name=f"G2_{isb}")
                for g in range(4):
                    et, _ = emit_group(f2a, fT2, ga, 8, 2 * g)
                    emit_pv(fT2, G, et, 8, 2 * g)
                emit_norm(G, w2, isb)

            def s4_all():
                ga = emit_ga(f4a, 0, b4[0], Q=N4)
                G = gg_pool.tile([C + 1, 512], dt.float32, tag="g", name="G4")
                et, _ = emit_group(f4a, fT4, ga, 2, 0, Q=N4)
                emit_pv(fT4, G, et, 2, 0, Q=N4)
                emit_norm(G, w4, 0, Q=N4)

            # sprinkle upsample chunks into sb4/sb5 streams (DVE has slack there)
            u4c = up4_chunks()
            for i, c in enumerate(u4c):
                prep_hooks[(4, 3 + i)] = c
            u2c = up2_chunks()
            for i, c in enumerate(u2c):
                prep_hooks[(5, 3 + i)] = c

            extras = {1: lambda: s2_sb(0), 2: lambda: s2_sb(1), 3: s4_all}
            for isb in range(1, 8):
                finish_warm(isb, w_next)
                sb1_part(isb, 2, 16)
                if isb < 7:
                    w_next = warm_sb(isb + 1)
                emit_norm(G1[isb], w1, isb)
                ex = extras.get(isb)
                if ex is not None:
                    ex()
                if isb == 6:
                    nc.vector.tensor_tensor(out_acc[:, 0:3584], out_acc[:, 0:3584],
                                            up_acc[:, 0:3584], op=ALU.add)
                    nc.sync.dma_start(out_d.ap()[:, 0:3584], out_acc[:, 0:3584])
                elif isb == 7:
                    nc.vector.tensor_tensor(out_acc[:, 3584:N1], out_acc[:, 3584:N1],
                                            up_acc[:, 3584:N1], op=ALU.add)
                    nc.sync.dma_start(out_d.ap()[:, 3584:N1], out_acc[:, 3584:N1])

            if debug:
                nc.sync.dma_start(dbg["d_f1a"].ap(), f1a[:])
                nc.sync.dma_start(dbg["d_fT1"].ap(), fT1[:])
                nc.sync.dma_start(dbg["d_f2a"].ap(), f2a[:])
                nc.sync.dma_start(dbg["d_fT2"].ap(), fT2[:])
                nc.sync.dma_start(dbg["d_f4a"].ap(), f4a[:])
                nc.sync.dma_start(dbg["d_att2"].ap(), att2p[:])
                nc.sync.dma_start(dbg["d_att4"].ap(), att4p[:])
                nc.sync.dma_start(dbg["d_upacc"].ap(), up_acc[:])

    nc.compile()
    return nc


def _prep_inputs(x, W_std):
    lap = np.array([[0., 1., 0.], [1., -4., 1.], [0., 1., 0.]], dtype=np.float32)
    Wl = W_std.astype(np.float32) + lap[None, None] * np.eye(C, dtype=np.float32)[:, :, None, None]
    # weight layout: cols [dy*64:(dy+1)*64] = paired taps (dy,0)|(dy,1) with K=128;
    # cols [(3+dy)*64:(4+dy)*64] = single tap (dy,2) on partitions 0..63.
    wt = np.zeros((128, 6 * C), dtype=np.float32)
    for dy in range(3):
        wt[0:C, dy * C:(dy + 1) * C] = Wl[:, :, dy, 0].T
        wt[C:128, dy * C:(dy + 1) * C] = Wl[:, :, dy, 1].T
        wt[0:C, (3 + dy) * C:(4 + dy) * C] = Wl[:, :, dy, 2].T
    B = x.shape[0]
    xps = np.zeros((B, 128, PAD, PAD), dtype=np.float32)
    xps[:, 0:C, 1:H + 1, 1:W + 1] = x
    xf = xps[:, 0:C].reshape(B, C, PAD * PAD)
    xps = xps.reshape(B, 128, PAD * PAD)
    xps[:, C:128, 0:PAD * PAD - 1] = xf[:, :, 1:]   # column-shifted copy
    # pooling matrices: P1 cols 0:32, P2 cols 32:64
    pp = np.zeros((128, 64), dtype=np.float32)
    for r in range(2):
        for c in range(64):
            pp[r * 64 + c, c // 2] = 0.25
    for r2 in range(4):
        for c2 in range(32):
            pp[r2 * 32 + c2, 32 + (r2 // 2) * 16 + c2 // 2] = 0.25
    import ml_dtypes
    bf = ml_dtypes.bfloat16
    return xps.astype(bf), wt.astype(bf), pp.astype(bf)


def _run(x, W_std, trace=False):
    x = np.asarray(x)
    W_std = np.asarray(W_std)
    xps, wt, pp = _prep_inputs(x, W_std)
    if "nc" not in _cache:
        _cache["nc"] = _build_nc()
    nc = _cache["nc"]
    in_maps = [{"xp": np.ascontiguousarray(xps[i]), "wt": wt, "pp": pp}
               for i in range(x.shape[0])]
    ncores = min(NCORES, x.shape[0])
    res = run_bass_kernel_spmd(nc, in_maps, core_ids=list(range(ncores)), trace=trace)
    out = np.stack([res.results[i]["out"].astype(np.float32).reshape(C, H, W)
                    for i in range(x.shape[0])])
    return out, res


def kernel(x, W_std):
    out, _ = _run(x, W_std, trace=False)
    return out
